# revision 2
# baseline (speedup 1.0000x reference)
"""CARAFE-downsample (K=5, stride=2) Trainium2 kernel v2, 8-core SPMD.

Key ideas vs baseline:
- Host de-interleaves x into 4 parity subgrids (fp16) so every
  stride-2 access (conv3x3 taps + all 25 reassembly taps) becomes a
  contiguous slice.
- Reassembly runs in fp16 (DVE 2x mode), masks broadcast as fp16.
- conv1x1 reads fp16 subgrids directly (full-rate PE matmuls).
- conv3x3 taps paired via a duplicated +1-col-shifted cx bank on
  partitions 64..127 (6 matmuls per 8-row block instead of 9).
Sharding: core = batch*2 + H-half, as baseline.
"""

import numpy as np

import concourse.bacc as bacc
import concourse.mybir as mybir
import concourse.tile as tile

F32 = mybir.dt.float32
F16 = mybir.dt.float16
AX = mybir.AxisListType
OP = mybir.AluOpType
ACTF = mybir.ActivationFunctionType

C, CC, H, W = 256, 64, 128, 128
B = 4
HO, WO = 32, 64           # per-core output dims
NPOS = HO * WO            # 2048
K5 = 5
GH, GW = 34, 66           # subgrid dims (rows, cols)
GSZ = GH * GW             # 2244 flat
NPC = 2                   # position chunks
PC = NPOS // NPC          # 1024 positions/chunk
TPC = 8                   # 128-pos mask tiles per chunk


def build_nc():
    nc = bacc.Bacc("TRN2", target_bir_lowering=False, debug=False)

    # inputs
    xq = nc.dram_tensor("xq", [C, 4, GH, GW], F16, kind="ExternalInput")
    w2a = nc.dram_tensor("w2a", [128, 128], F16, kind="ExternalInput")
    w2b = nc.dram_tensor("w2b", [128, 128], F16, kind="ExternalInput")
    wp = nc.dram_tensor("wp", [128, 3 * 41], F32, kind="ExternalInput")
    ws = nc.dram_tensor("ws", [64, 3 * 41], F32, kind="ExternalInput")
    ident = nc.dram_tensor("ident", [128, 128], F32, kind="ExternalInput")
    mscr = nc.dram_tensor("mscr", [25, NPOS], F16)
    y = nc.dram_tensor("y", [C, HO, WO], F32, kind="ExternalOutput")

    # conv3x3 tap schedule: 3 pairs (contract 128) + 3 singles (64)
    # pair j reads subgrid g at (hh0+dh, ww0) with real bank + shifted bank
    # pairs: ((0,0),(0,2))->g3,(dh 0); ((2,0),(2,2))->g3,(dh 1);
    #        ((1,0),(1,2))->g1,(dh 1)
    # singles: (0,1)->g2 @(0, +1); (2,1)->g2 @(1, +1); (1,1)->g0 @(1, +1)
    PAIRS = [(3, 0, 0), (3, 1, 0), (1, 1, 0)]   # (g, dh, dw)
    SINGLES = [(2, 0, 1), (2, 1, 1), (0, 1, 1)]

    with tile.TileContext(nc) as tc:
        with (
            tc.tile_pool(name="big", bufs=1) as bigpool,
            tc.tile_pool(name="work", bufs=3) as workpool,
            tc.tile_pool(name="tmp", bufs=8) as tmppool,
            tc.tile_pool(name="mbp", bufs=6) as mbpool,
            tc.tile_pool(name="ps", bufs=2, space="PSUM") as ps,
            tc.tile_pool(name="ps2", bufs=2, space="PSUM") as ps2,
            tc.tile_pool(name="ps3", bufs=2, space="PSUM") as ps3,
        ):
            # ---- persistent tiles ----
            x0 = bigpool.tile([128, 4, GH, GW], F16, tag="x0")
            x1 = bigpool.tile([128, 4, GH, GW], F16, tag="x1")
            w2as = bigpool.tile([128, 128], F16, tag="w2a")
            w2bs = bigpool.tile([128, 128], F16, tag="w2b")
            wps = bigpool.tile([128, 3 * 41], F32, tag="wp")
            wss = bigpool.tile([64, 3 * 41], F32, tag="ws")
            ids = bigpool.tile([128, 128], F32, tag="ident")
            cxd = bigpool.tile([128, 4, GH, GW], F32, tag="cxd")
            logits = bigpool.tile([41, NPOS], F32, tag="logits")
            mcm = bigpool.tile([25, NPOS], F16, tag="mcm")
            acc0 = bigpool.tile([128, HO, WO], F16, tag="acc0")
            acc1 = bigpool.tile([128, HO, WO], F16, tag="acc1")
            accs = [acc0, acc1]
            xs = [x0, x1]

            nc.sync.dma_start(out=w2as[:], in_=w2a[:])
            nc.sync.dma_start(out=w2bs[:], in_=w2b[:])
            nc.sync.dma_start(out=wps[:], in_=wp[:])
            nc.sync.dma_start(out=wss[:], in_=ws[:])
            nc.sync.dma_start(out=ids[:], in_=ident[:])
            # x loads split by (row-slab, subgrid, half) so conv starts early
            for (sa, sb) in ((0, 17), (17, 34)):
                for g in range(4):
                    nc.sync.dma_start(out=x0[:, g, sa:sb], in_=xq[0:128, g, sa:sb])
                    nc.sync.dma_start(out=x1[:, g, sa:sb], in_=xq[128:256, g, sa:sb])

            cxf = cxd.rearrange("p g h w -> p (g h w)")
            x0f = x0.rearrange("p g h w -> p (g h w)")
            x1f = x1.rearrange("p g h w -> p (g h w)")

            npool = [0]

            def reass_engine():
                # mults only: every 3rd to gpsimd
                npool[0] += 1
                return nc.gpsimd if npool[0] % 3 == 0 else nc.vector

            for pc in range(NPC):
                # conv3x3 for chunk pc reads subgrid rows:
                # chunk0 hh in [0,17), chunk1 hh in [16,34) -> compute
                # disjoint slabs [0,17) and [17,34)
                ra, rb = (0, 17) if pc == 0 else (17, 34)
                nrow = rb - ra

                # ---- conv1x1 on each subgrid slab ----
                for g in range(4):
                    o0 = g * GSZ + ra * GW
                    n = nrow * GW            # 1122 flat elems
                    CH1 = 512                # psum bank: <=512 f32/partition
                    for s0 in range(0, n, CH1):
                        m = min(CH1, n - s0)
                        pt = ps.tile([128, CH1], F32, tag="ps1")
                        nc.tensor.matmul(pt[:, 0:m], w2as[:],
                                         x0f[:, o0 + s0:o0 + s0 + m],
                                         start=True, stop=False)
                        nc.tensor.matmul(pt[:, 0:m], w2bs[:],
                                         x1f[:, o0 + s0:o0 + s0 + m],
                                         start=False, stop=True)
                        # copy real bank
                        nc.scalar.activation(cxf[0:64, o0 + s0:o0 + s0 + m],
                                             pt[0:64, 0:m], ACTF.Copy)
                        if g in (1, 3):
                            # shifted bank: dst[i] = cx[i+1]
                            d0 = o0 + s0 - 1
                            if d0 < g * GSZ:
                                nc.scalar.activation(
                                    cxf[64:128, d0 + 1:d0 + m],
                                    pt[64:128, 1:m], ACTF.Copy)
                            else:
                                nc.scalar.activation(
                                    cxf[64:128, d0:d0 + m],
                                    pt[64:128, 0:m], ACTF.Copy)

                # ---- conv3x3 (paired) -> logits ----
                for c4 in range(2 * pc, 2 * pc + 2):
                    hoc = 8 * c4
                    lgp = ps2.tile([41, 512], F32, tag="ps2")
                    nmm = 0
                    for j, (g, dh, dw) in enumerate(PAIRS):
                        rhs = cxd[:, g, hoc + dh: hoc + dh + 8, dw: dw + 64]
                        nc.tensor.matmul(lgp[:], wps[:, 41 * j: 41 * (j + 1)],
                                         rhs, start=(nmm == 0), stop=False)
                        nmm += 1
                    for j, (g, dh, dw) in enumerate(SINGLES):
                        rhs = cxd[0:64, g, hoc + dh: hoc + dh + 8, dw: dw + 64]
                        nc.tensor.matmul(lgp[:], wss[:, 41 * j: 41 * (j + 1)],
                                         rhs, start=False, stop=(j == 2))
                    nc.scalar.activation(logits[:, 512 * c4: 512 * (c4 + 1)],
                                         lgp[:], ACTF.Copy)

                # ---- transpose logits -> pos-major ----
                lgT = workpool.tile([128, TPC, 41], F32, tag="lgT")
                for tt in range(TPC):
                    t = TPC * pc + tt
                    tpp = ps3.tile([128, 41], F32, tag="ps3")
                    nc.tensor.transpose(tpp[:],
                                        logits[:, 128 * t: 128 * (t + 1)],
                                        ids[0:41, 0:41])
                    nc.scalar.activation(lgT[:, tt, :], tpp[:], ACTF.Copy)

                # ---- mask pipeline (pos-major) ----
                p8 = workpool.tile([128, TPC, 8], F32, tag="p8")
                nc.vector.tensor_tensor(p8[:], lgT[:, :, 25:33],
                                        lgT[:, :, 33:41], OP.mult)
                p4 = workpool.tile([128, TPC, 4], F32, tag="p4")
                nc.vector.tensor_tensor(p4[:], p8[:, :, 0:4], p8[:, :, 4:8],
                                        OP.mult)
                p2 = workpool.tile([128, TPC, 2], F32, tag="p2")
                nc.vector.tensor_tensor(p2[:], p4[:, :, 0:2], p4[:, :, 2:4],
                                        OP.mult)
                i0 = workpool.tile([128, TPC], F32, tag="i0")
                nc.vector.tensor_tensor(i0[:], p2[:, :, 0], p2[:, :, 1],
                                        OP.mult)
                ic = workpool.tile([128, TPC], F32, tag="ic")
                nc.vector.tensor_scalar(ic[:], i0[:], 10.0, -10.0,
                                        OP.min, OP.max)

                mskl = workpool.tile([128, TPC, 25], F32, tag="mskl")
                nc.vector.tensor_tensor(mskl[:], lgT[:, :, 0:25],
                                        ic[:].to_broadcast([128, TPC, 25]),
                                        OP.mult)
                tmax = workpool.tile([128, TPC], F32, tag="tmax")
                nc.vector.tensor_reduce(tmax[:], mskl[:], AX.X, OP.max)
                msub = workpool.tile([128, TPC, 25], F32, tag="msub")
                nc.vector.tensor_tensor(msub[:], mskl[:],
                                        tmax[:].to_broadcast([128, TPC, 25]),
                                        OP.subtract)
                mexp = workpool.tile([128, TPC, 25], F32, tag="mexp")
                nc.scalar.activation(mexp[:], msub[:], ACTF.Exp)
                msum = workpool.tile([128, TPC], F32, tag="msum")
                nc.vector.tensor_reduce(msum[:], mexp[:], AX.X, OP.add)
                mrec = workpool.tile([128, TPC], F32, tag="mrec")
                nc.vector.reciprocal(mrec[:], msum[:])
                mskn = workpool.tile([128, TPC, 25], F32, tag="mskn")
                nc.vector.tensor_tensor(mskn[:], mexp[:],
                                        mrec[:].to_broadcast([128, TPC, 25]),
                                        OP.mult)

                # ---- transpose mask back to channel-major (fp16) ----
                for tt in range(TPC):
                    t = TPC * pc + tt
                    mcp = ps3.tile([25, 128], F32, tag="ps3")
                    nc.tensor.transpose(mcp[:], mskn[:, tt, :], ids[:])
                    nc.scalar.activation(mcm[:, 128 * t: 128 * (t + 1)],
                                         mcp[:], ACTF.Copy)

                # ---- bounce mask chunk to DRAM for replicating DMAs ----
                nc.sync.dma_start(out=mscr[:, PC * pc: PC * (pc + 1)],
                                  in_=mcm[:, PC * pc: PC * (pc + 1)])

                # ---- reassembly for this chunk (software pipelined) ----
                ho0 = 16 * pc
                # issue all 25 broadcast DMAs up front (mbpool throttles)
                mbs = []
                for k in range(K5 * K5):
                    mb = mbpool.tile([128, PC], F16, tag="mb")
                    nc.sync.dma_start(
                        out=mb[:],
                        in_=mscr[k: k + 1,
                                 PC * pc: PC * (pc + 1)].to_broadcast(
                                     [128, PC]))
                    mbs.append(mb.rearrange("p (a b) -> p a b", a=16))

                def unit_aps(u):
                    k, ch = u // 2, u % 2
                    ky, kx = k // K5, k % K5
                    g = 2 * (ky % 2) + (kx % 2)
                    xsrc = xs[ch][:, g, ho0 + ky // 2: ho0 + ky // 2 + 16,
                                  kx // 2: kx // 2 + 64]
                    adst = accs[ch][:, ho0: ho0 + 16, :]
                    return xsrc, adst, mbs[k]

                NU = 50
                pool_set = {u for u in range(2, NU) if u % 5 in (1, 3)}
                tmps = {}
                LOOKAHEAD = 6
                for i in range(NU + 2):
                    if i == 0:
                        for up0 in sorted(u for u in pool_set
                                          if u < LOOKAHEAD):
                            xsrc, _, mbv = unit_aps(up0)
                            t = tmppool.tile([128, 16, WO], F16, tag="tp")
                            nc.gpsimd.tensor_tensor(t[:], xsrc, mbv[:],
                                                    OP.mult)
                            tmps[up0] = t
                    up = i + LOOKAHEAD
                    if up < NU and up in pool_set:
                        xsrc, _, mbv = unit_aps(up)
                        t = tmppool.tile([128, 16, WO], F16, tag="tp")
                        nc.gpsimd.tensor_tensor(t[:], xsrc, mbv[:], OP.mult)
                        tmps[up] = t
                    if i < NU:
                        if i < 2:
                            xsrc, adst, mbv = unit_aps(i)
                            nc.vector.tensor_tensor(adst, xsrc, mbv[:],
                                                    OP.mult)
                        elif i not in pool_set:
                            xsrc, _, mbv = unit_aps(i)
                            t = tmppool.tile([128, 16, WO], F16, tag="td")
                            nc.vector.tensor_tensor(t[:], xsrc, mbv[:],
                                                    OP.mult)
                            tmps[i] = t
                    j = i - 2
                    if j >= 2 and j in tmps:
                        _, adst, _ = unit_aps(j)
                        nc.vector.tensor_tensor(adst, adst, tmps.pop(j)[:],
                                                OP.add)
                # drain remaining adds (pool lookahead leftovers)
                for j in sorted(tmps):
                    _, adst, _ = unit_aps(j)
                    nc.vector.tensor_tensor(adst, adst, tmps[j][:], OP.add)
                tmps.clear()

                # ---- convert + store this chunk ----
                yf = y.rearrange("c h w -> c (h w)")
                for ch, acc in enumerate(accs):
                    y32 = workpool.tile([128, PC], F32, tag=f"y32_{ch}")
                    nc.scalar.activation(
                        y32[:],
                        acc.rearrange("p h w -> p (h w)")[:, PC * pc:
                                                          PC * (pc + 1)],
                        ACTF.Copy)
                    nc.scalar.dma_start(
                        out=yf[128 * ch: 128 * (ch + 1),
                               PC * pc: PC * (pc + 1)],
                        in_=y32[:])

    nc.finalize()
    return nc


def make_core_inputs(x, w_comp, b_comp, w_enc, b_enc, w_kenc, b_kenc):
    """Full inputs -> list of 8 per-core input dicts."""
    x = np.asarray(x)
    w_compT = np.ascontiguousarray(
        np.asarray(w_comp).reshape(CC, C).T).astype(np.float32)  # [256, 64]
    # conv1x1 stationaries: [w | w] duplicated out-cols, fp16
    w2a = np.concatenate([w_compT[0:128]] * 2, axis=1).astype(np.float16)
    w2b = np.concatenate([w_compT[128:256]] * 2, axis=1).astype(np.float16)

    we = np.asarray(w_enc)    # [25, 64, 3, 3]
    wk = np.asarray(w_kenc)   # [16, 64, 3, 3]
    w41 = np.concatenate([we, wk], axis=0)  # [41, 64, 3, 3]

    # pair stationaries [128, 3*41]; singles [64, 3*41]
    PAIR_TAPS = [((0, 0), (0, 2)), ((2, 0), (2, 2)), ((1, 0), (1, 2))]
    SINGLE_TAPS = [(0, 1), (2, 1), (1, 1)]
    wp = np.zeros((128, 3, 41), np.float32)
    for j, (ta, tb) in enumerate(PAIR_TAPS):
        wp[0:64, j] = w41[:, :, ta[0], ta[1]].T
        wp[64:128, j] = w41[:, :, tb[0], tb[1]].T
    wp = wp.reshape(128, 3 * 41)
    wss = np.zeros((64, 3, 41), np.float32)
    for j, (dy, dx) in enumerate(SINGLE_TAPS):
        wss[:, j] = w41[:, :, dy, dx].T
    wss = wss.reshape(64, 3 * 41)
    ident = np.eye(128, dtype=np.float32)

    maps = []
    for core in range(8):
        b, h = core // 2, core % 2
        start = 64 * h
        xpc = np.zeros((C, 68, 132), np.float32)
        lo, hi = start - 2, start + 66
        clo, chi = max(lo, 0), min(hi, H)
        xpc[:, clo - lo: clo - lo + (chi - clo), 2:130] = x[b, :, clo:chi, :]
        # de-interleave: g = 2*(row%2) + (col%2)
        xq = np.empty((C, 4, GH, GW), np.float16)
        xq[:, 0] = xpc[:, 0::2, 0::2]
        xq[:, 1] = xpc[:, 0::2, 1::2]
        xq[:, 2] = xpc[:, 1::2, 0::2]
        xq[:, 3] = xpc[:, 1::2, 1::2]
        maps.append({
            "xq": xq,
            "w2a": w2a,
            "w2b": w2b,
            "wp": wp,
            "ws": wss,
            "ident": ident,
        })
    return maps


def assemble_output(results):
    out = np.zeros((B, C, 64, 64), np.float32)
    for core in range(8):
        b, h = core // 2, core % 2
        out[b, :, 32 * h: 32 * (h + 1), :] = results[core]["y"]
    return out


_NC_CACHE = []


def kernel(**inputs):
    import numpy as _np
    from concourse.bass_utils import run_bass_kernel_spmd

    maps = make_core_inputs(
        inputs["x"], inputs["w_comp"], inputs["b_comp"], inputs["w_enc"],
        inputs["b_enc"], inputs["w_kenc"], inputs["b_kenc"])
    if not _NC_CACHE:
        _NC_CACHE.append(build_nc())
    res = run_bass_kernel_spmd(_NC_CACHE[0], maps, list(range(8)))
    out = assemble_output(res.results)
    return out.astype(_np.float32)
